# revision 1
# baseline (speedup 1.0000x reference)
"""Unfold/im2col kernel for Trainium2 (Bass/Tile), 8-core data parallel.

Problem: x [4, 64, 224, 224] f32 -> out [4, 576, 49729] f32 where
out[b, (c*3+kh)*3+kw, oh*223+ow] = pad(x,1)[b, c, oh+kh, ow+kw]
(3x3 kernel, pad 1, stride 1, dilation 1, oh=ow=223).

Sharding: 8 cores = (batch 4) x (channel half 2). Each core handles
32 channels -> [288, 49729] independently; outputs concatenate on the
channel axis (channel-major row layout makes halves contiguous).

The input is zero-padded host-side to [32, 226, 226] per core, so the
device kernel is pure DMA. All 32 padded images live in two SBUF tiles
(padded rows 0..127 / 128..225 on partitions, channels side by side in
the free dim), each filled by ONE load DMA. Each (kh, kw) window is
then written by one DMA per tile half per 16-channel block via a 3D
access pattern (window-row x channel x 223). Big stores issue on
gpsimd (SWDGE): its model-queue DMAs are spread across all 16 SDMA
engines (~230 GB/s at this 892 B descriptor size), whereas the HWDGE
dynamic rings feed a single SDMA engine (~15-28 GB/s) and only carry
the tiny split-remainder chunks. Measured ~308 us/core on TRN2
(roofline for 57 MB out + 6.5 MB in at ~358 GB/s HBM is ~180 us; the
892 B descriptor processing rate of the SDMA engines is the binding
limit).
"""

from contextlib import ExitStack

import numpy as np

import concourse.bass as bass
import concourse.tile as tile
from concourse import mybir
from concourse.ap import AP
from concourse.bass_utils import run_bass_kernel_spmd

B, C, IH, IW = 4, 64, 224, 224
N_CORES = 8
CPC = C // 2          # channels per core: 32
PH = IH + 2           # padded height/width: 226
OH = IH - 1           # output spatial: 223
OSZ = OH * OH         # 49729
NROW = CPC * 9        # 288 output rows per core
ROWS0 = 128           # padded rows 0..127 in tile0
ROWS1 = PH - ROWS0    # padded rows 128..225 in tile1 (98)
FREE = CPC * PH       # free dim elements per tile: 7232
PIMG = PH * PH        # padded image elements: 51076

_NC_CACHE = {}


def build_nc() -> bass.Bass:
    nc = bass.Bass()
    x = nc.declare_dram_parameter("xp", [CPC, PH, PH], mybir.dt.float32, isOutput=False)
    out = nc.declare_dram_parameter("out", [NROW, OSZ], mybir.dt.float32, isOutput=True)
    xb = x[:, :, :]
    ob = out[:, :]

    with tile.TileContext(nc) as tc:
        with ExitStack() as ctx:
            pool = ctx.enter_context(tc.tile_pool(name="img", bufs=1))
            t0 = pool.tile([ROWS0, FREE], mybir.dt.float32, name="t0", tag="t0")[:, :]
            t1 = pool.tile([ROWS1, FREE], mybir.dt.float32, name="t1", tag="t1")[:, :]

            # Two loads: tile partition p, free col c*226+w  <-  xp[c, p(+128), w]
            src0 = AP(xb.tensor, xb.offset,
                      [[PH, ROWS0], [PIMG, CPC], [1, PH]])
            dst0 = AP(t0.tensor, t0.offset,
                      [[FREE, ROWS0], [PH, CPC], [1, PH]])
            nc.gpsimd.dma_start(out=dst0, in_=src0)
            src1 = AP(xb.tensor, xb.offset + ROWS0 * PH,
                      [[PH, ROWS1], [PIMG, CPC], [1, PH]])
            dst1 = AP(t1.tensor, t1.offset,
                      [[FREE, ROWS1], [PH, CPC], [1, PH]])
            nc.gpsimd.dma_start(out=dst1, in_=src1)

            # Stores: for each (kh, kw), 16 channels per DMA (the channel
            # dim is split in half so the (window-row, channel, col) walk
            # keeps the partition-crossing step on dim 0 and no dim merge
            # fires; 32-channel and 4-channel variants measured slower).
            # out row (c*9 + kh*3 + kw), col r*223.. = padded[kh+r, kw..kw+222];
            # window rows 0..n0-1 live in tile0 (partitions kh..127), the rest
            # in tile1 (partitions 0..n1-1).
            # Row counts 97/113/127 crash the SWDGE path on device
            # (NRT_EXEC_UNIT_UNRECOVERABLE, found empirically), so split
            # those transfers into known-good chunk sizes.
            def safe_rows(n):
                if n in (128, 126, 124, 121, 120, 112, 96, 95, 64, 63, 31, 15, 1):
                    return [n]
                for first in (112, 96, 64):
                    if 0 < n - first and (n - first) in (63, 31, 15, 1):
                        return [first, n - first]
                return [n - 15, 15]

            # Each store: (kh, kw, h, tile, chunk-start-row r, rows n).
            CH2 = CPC // 2
            work = []
            for kh in range(3):
                n0 = ROWS0 - kh
                n1 = OH - n0
                for kw in range(3):
                    for h in range(2):
                        r = 0
                        for n in safe_rows(n0):
                            work.append((kh, kw, h, 0, r, n))
                            r += n
                        for n in safe_rows(n1):
                            work.append((kh, kw, h, 1, r, n))
                            r += n

            def emit(eng, kh, kw, h, tl, r, n):
                co = h * CH2
                if tl == 0:
                    src = AP(t0.tensor,
                             t0.offset + (kh + r) * FREE + co * PH + kw,
                             [[FREE, n], [PH, CH2], [1, OH]])
                else:
                    src = AP(t1.tensor,
                             t1.offset + (r - (ROWS0 - kh)) * FREE + co * PH + kw,
                             [[FREE, n], [PH, CH2], [1, OH]])
                dst = AP(ob.tensor,
                         ob.offset + (co * 9 + kh * 3 + kw) * OSZ + r * OH,
                         [[OH, n], [9 * OSZ, CH2], [1, OH]])
                eng.dma_start(out=dst, in_=src)

            # Tiny split-remainder chunks go to the (otherwise idle) HWDGE
            # queues; the big stores stay on the fast SWDGE model queue,
            # ordered tile0-first so the queue never stalls on load1.
            small = [w for w in work if w[5] <= 15]
            big = [w for w in work if w[5] > 15]
            for i, (kh, kw, h, tl, r, n) in enumerate(small):
                emit(nc.sync if i % 2 == 0 else nc.scalar, kh, kw, h, tl, r, n)
            for kh, kw, h, tl, r, n in sorted(big, key=lambda w: w[3]):
                emit(nc.gpsimd, kh, kw, h, tl, r, n)
    return nc


def _split_multi_waits(nc: bass.Bass) -> None:
    """Walrus allows only one sync-wait command per instruction (the
    kernel-tail drain ends up with one per DMA-completion sem lane).
    Hoist all but the last wait onto fresh single-wait NOPs inserted
    just before the instruction on the same engine — semantically
    identical (the engine blocks on each wait in turn)."""
    from bass_rust import SyncInfo

    k = 0
    for fn in nc.m.functions:
        for blk in fn.blocks:
            insts = blk.instructions
            for idx in range(len(insts) - 1, -1, -1):
                inst = insts[idx]
                si = inst.sync_info
                if si is None or len(si.on_wait) <= 1:
                    continue
                waits = list(si.on_wait)
                for w in waits[:-1]:
                    nop = mybir.InstNoOp(name=f"WSPLIT-{k}")
                    k += 1
                    nop.engine = inst.engine
                    nop.sync_info = SyncInfo(on_wait=[w], on_update=[])
                    insts.insert(idx, nop)
                si.on_wait = [waits[-1]]
                inst.sync_info = si


def get_nc() -> bass.Bass:
    if "nc" not in _NC_CACHE:
        nc = build_nc()
        _split_multi_waits(nc)
        _NC_CACHE["nc"] = nc
    return _NC_CACHE["nc"]


def make_in_maps(x: np.ndarray) -> list[dict]:
    x = np.asarray(x, dtype=np.float32)
    xp = np.pad(x, ((0, 0), (0, 0), (1, 1), (1, 1)))
    maps = []
    for core in range(N_CORES):
        b, half = divmod(core, 2)
        maps.append({"xp": np.ascontiguousarray(xp[b, half * CPC:(half + 1) * CPC])})
    return maps


def gather_out(results: list[dict]) -> np.ndarray:
    out = np.empty((B, C * 9, OSZ), dtype=np.float32)
    for core in range(N_CORES):
        b, half = divmod(core, 2)
        out[b, half * NROW:(half + 1) * NROW] = results[core]["out"]
    return out


def kernel(**inputs) -> np.ndarray:
    x = inputs["x"]
    nc = get_nc()
    res = run_bass_kernel_spmd(nc, make_in_maps(x), list(range(N_CORES)))
    return gather_out(res.results)



# revision 8
# speedup vs baseline: 1.0908x; 1.0908x over previous
"""Unfold/im2col kernel for Trainium2 (Bass/Tile), 8-core data parallel.

Problem: x [4, 64, 224, 224] f32 -> out [4, 576, 49729] f32 where
out[b, (c*3+kh)*3+kw, oh*223+ow] = pad(x,1)[b, c, oh+kh, ow+kw]
(3x3 kernel, pad 1, stride 1, dilation 1, oh=ow=223).

Sharding: 8 cores = (batch 4) x (channel half 2). Each core handles
32 channels -> [288, 49729] independently; outputs concatenate on the
channel axis (channel-major row layout makes halves contiguous).

Layout strategy (v2): the old kernel stored straight from a
row-partitioned image, which caps every store descriptor at one
223-element window row (892 B) -- the SDMA engines' per-descriptor
overhead then limits aggregate BW (308 us measured). Here the padded
input is loaded with partition = (row-block, channel): each of the 128
partitions holds 58 padded rows x 226 of one channel (4 blocks of 56
output rows cover oh=223 with a 2-row halo). The scalar (Activation)
and vector (DVE) engines then materialize each of the 9 (kh, kw)
shifted windows into a staging tile where output rows are contiguous
per partition. Stores then move CROW output rows per descriptor.

Descriptor sizing: loads are split into separate chunk DMAs; staging
chunks are separated by a 1-element gap so the DMA AP optimizer cannot
re-merge them into >CROW-row descriptors (the symbolic-AP lowering
path ignores max_dma_last_dim).
"""

from contextlib import ExitStack

import numpy as np

import concourse.bass as bass
import concourse.tile as tile
from concourse import mybir
from concourse.ap import AP
from concourse.bass_utils import run_bass_kernel_spmd

B, C, IH, IW = 4, 64, 224, 224
N_CORES = 8
CPC = C // 2          # channels per core: 32
PH = IH + 2           # padded height/width: 226
OH = IH - 1           # output spatial: 223
OSZ = OH * OH         # 49729
NROW = CPC * 9        # 288 output rows per core
PIMG = PH * PH        # padded image elements: 51076
NBLK = 4              # output-row blocks (partition = blk*32 + c)
BSTEP = 56            # output rows per block (last block: 55)
BR = BSTEP + 2        # padded rows loaded per partition: 58
FIMG = BR * PH        # input free elems per partition: 13108
NB = 2                # staging buffers

LROWS = 29            # padded rows per load chunk DMA (29*226*4 = 26216 B)
CROW = 14             # staged rows per store chunk     (14*223*4 = 12488 B)

NCH = -(-BSTEP // CROW)        # chunks per window per partition
CHST = CROW * OH + 1           # gapped chunk stride (elems)
SFREE = NCH * CHST             # staging free elems per partition

_NC_CACHE = {}


def build_nc() -> bass.Bass:
    nc = bass.Bass()
    x = nc.declare_dram_parameter("xp", [CPC, PH, PH], mybir.dt.float32, isOutput=False)
    out = nc.declare_dram_parameter("out", [NROW, OSZ], mybir.dt.float32, isOutput=True)
    xb = x[:, :, :]
    ob = out[:, :]

    with tile.TileContext(nc) as tc:
        with ExitStack() as ctx:
            pool = ctx.enter_context(tc.tile_pool(name="p", bufs=1))
            timg = pool.tile([NBLK * CPC, FIMG], mybir.dt.float32,
                             name="timg", tag="timg")[:, :]
            stg = [pool.tile([NBLK * CPC, SFREE], mybir.dt.float32,
                             name=f"stg{i}", tag=f"stg{i}")[:, :] for i in range(NB)]

            # Loads: partition blk*32+c <- xp[c, blk*56 : blk*56+58, :],
            # chunked into LROWS-row DMAs to bound descriptor size.
            for blk in range(NBLK):
                r = 0
                while r < BR:
                    n = min(LROWS, BR - r)
                    src = AP(xb.tensor, xb.offset + (blk * BSTEP + r) * PH,
                             [[PIMG, CPC], [1, n * PH]])
                    dst = AP(timg.tensor,
                             timg.offset + blk * CPC * FIMG + r * PH,
                             [[FIMG, CPC], [1, n * PH]])
                    nc.gpsimd.dma_start(out=dst, in_=src)
                    r += n

            # For each (kh, kw): engine-copy the shifted window into staging
            # (row r of the block -> padded row kh+r, cols kw..kw+222; same
            # local offsets in every partition), one copy per CROW-row chunk,
            # then store one DMA per block (+1 for blk3's short tail).
            for w in range(9):
                kh, kw = divmod(w, 3)
                s = stg[w % NB]
                eng = nc.scalar if w % 2 == 0 else nc.vector
                for ch in range(NCH):
                    csrc = AP(timg.tensor,
                              timg.offset + (kh + ch * CROW) * PH + kw,
                              [[FIMG, NBLK * CPC], [PH, CROW], [1, OH]])
                    cdst = AP(s.tensor, s.offset + ch * CHST,
                              [[SFREE, NBLK * CPC], [OH, CROW], [1, OH]])
                    if w % 2 == 0:
                        eng.copy(out=cdst, in_=csrc)
                    else:
                        eng.tensor_copy(cdst, csrc)
                for blk in range(NBLK):
                    nv = min(BSTEP, OH - blk * BSTEP)   # 56,56,56,55
                    nfull, rem = divmod(nv, CROW)
                    base_s = s.offset + blk * CPC * SFREE
                    base_d = (ob.offset + (kh * 3 + kw) * OSZ
                              + blk * BSTEP * OH)
                    ssrc = AP(s.tensor, base_s,
                              [[SFREE, CPC], [CHST, nfull], [1, CROW * OH]])
                    sdst = AP(ob.tensor, base_d,
                              [[9 * OSZ, CPC], [CROW * OH, nfull], [1, CROW * OH]])
                    nc.gpsimd.dma_start(out=sdst, in_=ssrc)
                    if rem:
                        ssrc = AP(s.tensor, base_s + nfull * CHST,
                                  [[SFREE, CPC], [1, rem * OH]])
                        sdst = AP(ob.tensor, base_d + nfull * CROW * OH,
                                  [[9 * OSZ, CPC], [1, rem * OH]])
                        nc.gpsimd.dma_start(out=sdst, in_=ssrc)
    return nc


def _split_multi_waits(nc: bass.Bass) -> None:
    """Walrus allows only one sync-wait command per instruction (the
    kernel-tail drain ends up with one per DMA-completion sem lane).
    Hoist all but the last wait onto fresh single-wait NOPs inserted
    just before the instruction on the same engine — semantically
    identical (the engine blocks on each wait in turn)."""
    from bass_rust import SyncInfo

    k = 0
    for fn in nc.m.functions:
        for blk in fn.blocks:
            insts = blk.instructions
            for idx in range(len(insts) - 1, -1, -1):
                inst = insts[idx]
                si = inst.sync_info
                if si is None or len(si.on_wait) <= 1:
                    continue
                waits = list(si.on_wait)
                for w in waits[:-1]:
                    nop = mybir.InstNoOp(name=f"WSPLIT-{k}")
                    k += 1
                    nop.engine = inst.engine
                    nop.sync_info = SyncInfo(on_wait=[w], on_update=[])
                    insts.insert(idx, nop)
                si.on_wait = [waits[-1]]
                inst.sync_info = si


def get_nc() -> bass.Bass:
    if "nc" not in _NC_CACHE:
        nc = build_nc()
        _split_multi_waits(nc)
        _NC_CACHE["nc"] = nc
    return _NC_CACHE["nc"]


def make_in_maps(x: np.ndarray) -> list[dict]:
    x = np.asarray(x, dtype=np.float32)
    xp = np.pad(x, ((0, 0), (0, 0), (1, 1), (1, 1)))
    maps = []
    for core in range(N_CORES):
        b, half = divmod(core, 2)
        maps.append({"xp": np.ascontiguousarray(xp[b, half * CPC:(half + 1) * CPC])})
    return maps


def gather_out(results: list[dict]) -> np.ndarray:
    out = np.empty((B, C * 9, OSZ), dtype=np.float32)
    for core in range(N_CORES):
        b, half = divmod(core, 2)
        out[b, half * NROW:(half + 1) * NROW] = results[core]["out"]
    return out


def kernel(**inputs) -> np.ndarray:
    x = inputs["x"]
    nc = get_nc()
    res = run_bass_kernel_spmd(nc, make_in_maps(x), list(range(N_CORES)))
    return gather_out(res.results)
